# revision 1
# baseline (speedup 1.0000x reference)
"""FAMoE layer Trainium2 kernel (v3).

Per batch row j of x [B, H, L] (pure data parallel, B/8 rows per core):

  Input DMA (E-layout): h = 512*s + 4*p + q -> x_nat[p, (b, s, q, l)] bf16,
  descriptors read 4 consecutive h-rows (800 B) -> full-rate HBM streaming.

  8 narrow PE transposes ([128, 64] stationary) -> p_xt; DVE copy (2x bf16)
  -> xt [114, 512]: rows (s, l) at 0-49/64-113, cols (q, p).

  fwd DFT p_cs = ri2^T @ xt; gating via ACT Square (h-subsampled by SS),
  PE fold (C^2+S^2, quad-batched), ACT Sqrt+accum -> gbuf.

  Gating MLP per chunk of NB rows -> wrep [116, NB].

  Inverse without a cs copy: out = (ri2c @ lo_j)^T @ xt. lo (DVE 2x
  tensor_scalar) and m = ri2c @ lo are quad-batched; the inverse reads the
  kept xt tiles. Output cols are permuted (q, p) -> c = 4p + q inside the
  mandatory PSUM->SBUF copy so h = 512*s + c is DRAM-contiguous (2 KB
  descriptors).

  The main loop interleaves chunk k+1's forward phase with chunk k's
  inverse phase at j granularity so no engine queue blocks on a
  cross-engine round trip.
"""

import sys

sys.path.insert(0, "/opt/trn_rl_repo")

import numpy as np

N_CORES = 8
B, H, L = 2048, 1024, 50
F = 26
E = 8
BS = B // N_CORES          # 256 batch rows per core
NB = 32                    # gating chunk size
GRP = 8                    # batch rows per input DMA group
OJ = 4                     # batch rows per output DMA group
QJ = 4                     # quad-batch for fold / m matmuls
SS = 1                     # gating h-subsample stride (1 = exact, required)

_cache = {}


def _dft_consts():
    l = np.arange(L)[:, None].astype(np.float64)
    f = np.arange(F)[None, :].astype(np.float64)
    ang = 2.0 * np.pi * l * f / L
    R = np.cos(ang)
    I = -np.sin(ang)
    c = np.full(F, 2.0)
    c[0] = 1.0
    c[F - 1] = 1.0
    A = c[None, :] * np.cos(ang) / L
    B2 = -c[None, :] * np.sin(ang) / L
    return R, I, A, B2


def _build_ri2(R, I):
    RI2 = np.zeros((114, 128), np.float64)
    RI2[0:50, 0:26] = R
    RI2[64:114, 26:52] = R
    RI2[0:50, 64:90] = I
    RI2[64:114, 90:116] = I
    return RI2


def _build_abb(A, B2):
    ABB = np.zeros((116, 128), np.float64)
    ABB[0:26, 0:50] = A.T
    ABB[26:52, 64:114] = A.T
    ABB[64:90, 0:50] = B2.T
    ABB[90:116, 64:114] = B2.T
    return ABB


def _build_fold():
    Fm = np.zeros((116, 52), np.float32)
    Fm[np.arange(52), np.arange(52)] = 1.0
    Fm[64 + np.arange(52), np.arange(52)] = 1.0
    return Fm


def _build_sel():
    S = np.zeros((26, 116), np.float32)
    for base in (0, 26, 64, 90):
        S[np.arange(26), base + np.arange(26)] = 1.0
    return S


def _build_program(bs, nb):
    from concourse import bacc, bass, mybir, tile

    f32 = mybir.dt.float32
    bf16 = mybir.dt.bfloat16

    nc = bacc.Bacc("TRN2", target_bir_lowering=False, debug=False)

    x_d = nc.dram_tensor("x", [bs, H, L], f32, kind="ExternalInput")
    out_d = nc.dram_tensor("out", [bs, L, H], f32, kind="ExternalOutput")
    ident_d = nc.dram_tensor("ident", [128, 128], bf16, kind="ExternalInput")
    ri2_d = nc.dram_tensor("ri2", [114, 128], bf16, kind="ExternalInput")
    ri2ct_d = nc.dram_tensor("ri2ct", [116, 114], bf16, kind="ExternalInput")
    abb_d = nc.dram_tensor("abb", [116, 128], bf16, kind="ExternalInput")
    fold_d = nc.dram_tensor("fold", [116, 52], bf16, kind="ExternalInput")
    sel_d = nc.dram_tensor("sel", [F, 116], f32, kind="ExternalInput")
    w1f_d = nc.dram_tensor("w1f", [2 * F, F], f32, kind="ExternalInput")
    b1_d = nc.dram_tensor("b1c", [F, 1], f32, kind="ExternalInput")
    w2_d = nc.dram_tensor("w2", [F, E], f32, kind="ExternalInput")
    b2_d = nc.dram_tensor("b2c", [E, 1], f32, kind="ExternalInput")
    mask_d = nc.dram_tensor("mask", [E, F], f32, kind="ExternalInput")
    ones8_d = nc.dram_tensor("ones8", [E, 1], f32, kind="ExternalInput")
    ones8r_d = nc.dram_tensor("ones8r", [1, E], f32, kind="ExternalInput")

    n_chunk = bs // nb
    ngrp = nb // GRP
    W = 512 // SS

    with tile.TileContext(nc) as tc:
        with (
            tc.tile_pool(name="consts", bufs=1) as cpool,
            tc.tile_pool(name="xin", bufs=2 * ngrp + 1) as xpool,
            tc.tile_pool(name="xt", bufs=2 * nb + 2) as xtpool,
            tc.tile_pool(name="sq", bufs=3) as sqpool,
            tc.tile_pool(name="mag", bufs=2) as magpool,
            tc.tile_pool(name="mw", bufs=2) as mwpool,
            tc.tile_pool(name="gat", bufs=2) as gpool,
            tc.tile_pool(name="osb", bufs=2) as opool,
            tc.tile_pool(name="ps_xt", bufs=2, space="PSUM") as ps_xt,
            tc.tile_pool(name="ps_cs", bufs=2, space="PSUM") as ps_cs,
            tc.tile_pool(name="ps_m", bufs=1, space="PSUM") as ps_m,
            tc.tile_pool(name="ps_f", bufs=1, space="PSUM") as ps_f,
            tc.tile_pool(name="ps_o", bufs=2, space="PSUM") as ps_o,
        ):
            ident = cpool.tile([128, 128], bf16)
            ri2 = cpool.tile([114, 128], bf16)
            ri2ct = cpool.tile([116, 114], bf16)
            abb = cpool.tile([116, 128], bf16)
            fold = cpool.tile([116, 52], bf16)
            sel = cpool.tile([F, 116], f32)
            w1f = cpool.tile([2 * F, F], f32)
            b1 = cpool.tile([F, 1], f32)
            w2 = cpool.tile([F, E], f32)
            b2 = cpool.tile([E, 1], f32)
            mask = cpool.tile([E, F], f32)
            ones8 = cpool.tile([E, 1], f32)
            ones8r = cpool.tile([1, E], f32)
            for t, d in [
                (ident, ident_d), (ri2, ri2_d), (ri2ct, ri2ct_d),
                (abb, abb_d), (fold, fold_d), (sel, sel_d), (w1f, w1f_d),
                (b1, b1_d), (w2, w2_d), (b2, b2_d), (mask, mask_d),
                (ones8, ones8_d), (ones8r, ones8r_d),
            ]:
                nc.sync.dma_start(t[:], d[:])

            Sq = mybir.ActivationFunctionType.Square
            Sqrt = mybir.ActivationFunctionType.Sqrt
            Copy = mybir.ActivationFunctionType.Copy
            Relu = mybir.ActivationFunctionType.Relu
            Exp = mybir.ActivationFunctionType.Exp
            MUL = mybir.AluOpType.mult

            def emit_group_dma(g):
                x_nat = xpool.tile([128, GRP * 400 + 64], bf16, tag="xnat")
                bb = g * GRP
                src = x_d[bb : bb + GRP].rearrange(
                    "b (s p q) l -> p b s (q l)", s=2, p=128, q=4
                )
                dst = x_nat[:, 0 : GRP * 400].rearrange(
                    "p (b s ql) -> p b s ql", b=GRP, s=2, ql=200
                )
                nc.gpsimd.dma_start(out=dst, in_=src)
                nc.gpsimd.memset(x_nat[:, GRP * 400 : GRP * 400 + 64], 0.0)
                return x_nat

            # per-chunk state: x_nat group tiles, xt tiles, gbuf, wrep
            st = {}

            def fwd_transp(c, j):
                s_ = st[c]
                x_nat = s_["x_nats"][j // GRP]
                jl = j % GRP
                jh0 = j % 2
                if jh0 == 0:
                    s_["pxt2"] = ps_xt.tile(
                        [128, 1024], bf16, tag="pxt", name="pxt2"
                    )
                p_xt = s_["pxt2"]
                for s in range(2):
                    for q in range(4):
                        off = jl * 400 + s * 200 + q * 50
                        nc.tensor.matmul(
                            p_xt[64 * s : 64 * s + 64,
                                 512 * jh0 + 128 * q : 512 * jh0 + 128 * q + 128],
                            x_nat[:, off : off + 64],
                            ident[:],
                            is_transpose=True,
                        )
                xt = xtpool.tile([114, 512], bf16, tag="xt", name="xt")
                nc.vector.tensor_copy(
                    xt[:], p_xt[0:114, 512 * jh0 : 512 * jh0 + 512]
                )
                s_["xts"].append(xt[:])

            def fwd_dft(c, j):
                s_ = st[c]
                p_cs = ps_cs.tile([128, 512], f32, tag="pcs", name="pcs")
                nc.tensor.matmul(p_cs[:], ri2[:], s_["xts"][j])
                s_["pcs"][j] = p_cs

            def fwd_square(c, j):
                s_ = st[c]
                sq = sqpool.tile([116, 512], bf16, tag="sq", name="sq")
                nc.scalar.activation(sq[:], s_["pcs"][j][0:116, :], Sq)
                del s_["pcs"][j]
                s_["sq"][j] = sq

            def fwd_fold(c, j):
                s_ = st[c]
                p_fold = ps_f.tile([52, 512], f32, tag="sm")
                nc.tensor.matmul(p_fold[:], fold[:], s_["sq"][j][:])
                del s_["sq"][j]
                mag = magpool.tile([52, 512], bf16, tag="mag")
                nc.scalar.activation(
                    mag[:], p_fold[:], Sqrt,
                    accum_out=s_["gbuf"][:, j : j + 1],
                )

            FLAG = 6  # total forward-pipeline depth (drain steps)

            def fwd_step(c, t):
                if t < nb:
                    fwd_transp(c, t)
                if 2 <= t <= nb + 1:
                    fwd_dft(c, t - 2)
                if 4 <= t <= nb + 3:
                    fwd_square(c, t - 4)
                if 6 <= t <= nb + 5:
                    fwd_fold(c, t - 6)

            def emit_mlp(c):
                s_ = st[c]
                gbuf = s_["gbuf"]
                p_h1 = ps_f.tile([F, nb], f32, tag="sm")
                nc.tensor.matmul(p_h1[:], w1f[:], gbuf[:])
                h1 = gpool.tile([F, nb], f32, tag="h1")
                nc.scalar.activation(h1[:], p_h1[:], Relu, bias=b1[:])
                p_z = ps_f.tile([E, nb], f32, tag="sm")
                nc.tensor.matmul(p_z[:], w2[:], h1[:])
                ez = gpool.tile([E, nb], f32, tag="ez")
                nc.scalar.activation(ez[:], p_z[:], Exp, bias=b2[:])
                p_s = ps_f.tile([1, nb], f32, tag="sm")
                nc.tensor.matmul(p_s[:], ones8[:], ez[:])
                rs = gpool.tile([1, nb], f32, tag="rs")
                nc.vector.reciprocal(rs[:], p_s[:])
                p_r8 = ps_f.tile([E, nb], f32, tag="sm")
                nc.tensor.matmul(p_r8[:], ones8r[:], rs[:])
                ezn = gpool.tile([E, nb], f32, tag="ezn")
                nc.vector.tensor_tensor(ezn[:], ez[:], p_r8[:], MUL)
                p_w = ps_f.tile([F, nb], f32, tag="sm")
                nc.tensor.matmul(p_w[:], mask[:], ezn[:])
                w_sb = gpool.tile([F, nb], f32, tag="wsb")
                nc.vector.tensor_copy(w_sb[:], p_w[:])
                p_wrep = ps_f.tile([116, nb], f32, tag="sm")
                nc.tensor.matmul(p_wrep[:], sel[:], w_sb[:])
                wrep = gpool.tile([116, nb], f32, tag="wrep")
                nc.vector.tensor_copy(wrep[:], p_wrep[:])
                s_["wrep"] = wrep

            def emit_inv_j(c, j):
                s_ = st[c]
                bb = c * nb + j
                jq = j % QJ
                g = j // QJ
                wrep = s_["wrep"]
                # build lo / m / msb for quad g+1 while running quad g
                jn = j + QJ
                if jn < nb:
                    if jq == 0:
                        s_["loq"] = mwpool.tile(
                            [116, QJ * 128], bf16, tag="lo", name="loq"
                        )
                    nc.vector.tensor_scalar(
                        s_["loq"][:, jq * 128 : jq * 128 + 128], abb[:],
                        wrep[:, jn : jn + 1], None, MUL,
                    )
                    if jq == QJ - 1:
                        emit_mquad(s_, g + 1)
                # inverse + out-copy for row j (msbq computed a quad ahead)
                msbq = s_["msbq"][g]
                if jq == 0:
                    s_["osb"] = opool.tile(
                        [114, OJ * 512], f32, tag="osb", name="osb"
                    )
                osb = s_["osb"]
                p_o = ps_o.tile([128, 512], f32, tag="po")
                nc.tensor.matmul(
                    p_o[:], msbq[:, jq * 128 : jq * 128 + 128], s_["xts"][j]
                )
                dstv = osb[:, 512 * jq : 512 * jq + 512]
                srcv = p_o[0:114, :].rearrange("r (q p) -> r p q", q=4, p=128)
                nc.vector.tensor_copy(dstv, srcv)
                if jq == QJ - 1:
                    b0 = c * nb + j - OJ + 1
                    nc.sync.dma_start(
                        out=out_d[b0 : b0 + OJ, :, 0:512].rearrange(
                            "b l n -> l b n"
                        ),
                        in_=osb[0:50, :].rearrange(
                            "l (b n) -> l b n", b=OJ, n=512
                        ),
                    )
                    nc.sync.dma_start(
                        out=out_d[b0 : b0 + OJ, :, 512:1024].rearrange(
                            "b l n -> l b n"
                        ),
                        in_=osb[64:114, :].rearrange(
                            "l (b n) -> l b n", b=OJ, n=512
                        ),
                    )

            def emit_mquad(s_, g):
                p_mq = ps_m.tile([114, QJ * 128], f32, tag="pm")
                nc.tensor.matmul(p_mq[:], ri2ct[:], s_["loq"][:])
                msbq = mwpool.tile([114, QJ * 128], bf16, tag="msb", name="msbq")
                nc.vector.tensor_copy(msbq[:], p_mq[:])
                s_["msbq"][g] = msbq

            def emit_inv_prologue(c):
                # lo / m / msb for quad 0 of chunk c
                s_ = st[c]
                wrep = s_["wrep"]
                s_["msbq"] = {}
                s_["loq"] = mwpool.tile(
                    [116, QJ * 128], bf16, tag="lo", name="loq"
                )
                for i in range(QJ):
                    nc.vector.tensor_scalar(
                        s_["loq"][:, i * 128 : i * 128 + 128], abb[:],
                        wrep[:, i : i + 1], None, MUL,
                    )
                emit_mquad(s_, 0)

            def new_chunk_state(c):
                st[c] = {
                    "x_nats": None,
                    "xts": [],
                    "pcs": {},
                    "sq": {},
                    "gbuf": gpool.tile([52, nb], f32, tag="gbuf", name="gbuf"),
                }

            # ---------------- prologue ----------------
            new_chunk_state(0)
            st[0]["x_nats"] = [emit_group_dma(g) for g in range(ngrp)]
            if n_chunk > 1:
                new_chunk_state(1)
                st[1]["x_nats"] = [
                    emit_group_dma(ngrp + g) for g in range(ngrp)
                ]
            for t in range(nb + 6):
                fwd_step(0, t)

            # ---------------- main: inv(k) || fwd(k+1) ----------------
            for k in range(n_chunk):
                emit_mlp(k)
                emit_inv_prologue(k)
                if k + 2 < n_chunk:
                    new_chunk_state(k + 2)
                    st[k + 2]["x_nats"] = [
                        emit_group_dma((k + 2) * ngrp + g)
                        for g in range(ngrp)
                    ]
                for t in range(nb + 6):
                    if k + 1 < n_chunk:
                        fwd_step(k + 1, t)
                    if t < nb:
                        emit_inv_j(k, t)
                del st[k]

    nc.compile()
    return nc


def _get_program(bs=BS, nb=NB):
    key = (bs, nb)
    if key not in _cache:
        _cache[key] = _build_program(bs, nb)
    return _cache[key]


def _host_consts(band_boundaries, W1, b1, W2, b2):
    import ml_dtypes

    bf = ml_dtypes.bfloat16
    R, I, A, B2 = _dft_consts()
    sig = 1.0 / (1.0 + np.exp(-band_boundaries.astype(np.float64)))
    bounds = np.concatenate([[0.0], np.sort(sig), [1.0]])
    idx = (bounds * F).astype(np.int32)
    idx[-1] = F
    k = np.arange(F)
    mask = (
        (k[None, :] >= idx[:-1, None]) & (k[None, :] < idx[1:, None])
    ).astype(np.float32)
    ri2 = _build_ri2(R, I)
    w1f = np.concatenate([W1, W1], axis=0).astype(np.float64) * (SS / 1024.0)
    return {
        "ident": np.eye(128, dtype=np.float32).astype(bf),
        "ri2": ri2.astype(np.float32).astype(bf),
        "ri2ct": ri2[:, 0:116].T.astype(np.float32).astype(bf),
        "abb": _build_abb(A, B2).astype(np.float32).astype(bf),
        "fold": _build_fold().astype(bf),
        "sel": _build_sel(),
        "w1f": w1f.astype(np.float32),
        "b1c": b1.reshape(F, 1).astype(np.float32),
        "w2": W2.astype(np.float32),
        "b2c": b2.reshape(E, 1).astype(np.float32),
        "mask": mask,
        "ones8": np.ones((E, 1), np.float32),
        "ones8r": np.ones((1, E), np.float32),
    }


def kernel(x, band_boundaries, W1, b1, W2, b2):
    from concourse.bass_utils import run_bass_kernel_spmd

    nc = _get_program()
    consts = _host_consts(
        np.asarray(band_boundaries), np.asarray(W1), np.asarray(b1),
        np.asarray(W2), np.asarray(b2),
    )
    x = np.ascontiguousarray(np.asarray(x, dtype=np.float32))
    in_maps = [
        {"x": x[i * BS : (i + 1) * BS], **consts} for i in range(N_CORES)
    ]
    res = run_bass_kernel_spmd(nc, in_maps, list(range(N_CORES)))
    return np.concatenate([res.results[i]["out"] for i in range(N_CORES)], axis=0)

